# revision 10
# baseline (speedup 1.0000x reference)
"""
Trainium2 Bass kernel for nn_MetaAttention.

Computation (per batch b):
    rowsum[h,i]     = sum_j m[b,h,i,j]
    aggregated[i,j] = sum_h rowsum[h,i] * m[b,h,i,j]
    out[b]          = softmax(aggregated.flatten()).reshape(N, N)

Sharding: pure data parallel over B=16 across 8 cores (2 batches/core).

v2 design (DMA streams at the 358 GB/s/core HBM peak; goal = keep every
engine under the ~172 us stream time and shrink the end-of-kernel tail):
  - Row layout: 128 partitions x 6 contiguous rows (row = 6p + s);
    remainder rows 768..783 as [16, 784].
  - Scale-accumulate: slots 0-2 on PE via diag(rowsum) matmuls (f32,
    accumulating in PSUM over all 12 heads; the softmax chain's small
    PE outputs allocate from the same ring after the per-slot exp
    frees it); slots 3-5 via DVE scalar_tensor_tensor.
  - Rowsums: ACT copy+accum_out for 4 of 6 head-pairs + all remainder
    rows, DVE tensor_reduce for the rest. diag builds on DVE
    tensor_scalar (~130ns).
  - Remainder scale-accumulate on the otherwise idle GPSIMD engine.
  - Online softmax (per-slot negated max + exp + fused sum, global
    correction folded into the final scale).
  - Tail: last head of the last batch streams as [2,2,1,1]-slot pieces
    with finalize immediately per slot; partial global-min over slots
    0-4 + remainder precomputed; final stores alternate sync/scalar
    DMA queues so the two rings drain in parallel.
  - Output stored as bf16 (host upcasts; rel tol 2e-2).
"""

import numpy as np

B, H, N = 16, 12, 784
NCORES = 8
BPC = B // NCORES          # batches per core
P = 128                    # partitions
SLOTS = 6                  # full row-slots: row = 6p + s (rows 0..767)
REMP = 16                  # remainder rows 768..783 on partitions 0..15
PE_SLOTS = (0, 1, 2)       # slots accumulated on PE/PSUM
SB_SLOTS = (3, 4, 5)       # slots accumulated on DVE stt
JSPLITS = [(0, 512), (512, 272)]   # matmul free-dim splits (bank aligned)
RS_ACT_PAIRS = (0, 1, 2, 4)  # head-pairs whose rowsum runs on ACT
MM_F32R = False            # fp32r needs pre-rounded inputs; keep f32

LAST_RESULT = None  # BassKernelResults of the most recent kernel() call


def build_program():
    import concourse.bacc as bacc
    import concourse.tile as tile
    from concourse import mybir

    f32 = mybir.dt.float32
    f32r = mybir.dt.float32r
    bf16 = mybir.dt.bfloat16
    AX = mybir.AxisListType.X
    OP = mybir.AluOpType
    AF = mybir.ActivationFunctionType
    nc = bacc.Bacc("TRN2")

    def mmview(ap):
        return ap.bitcast(f32r) if MM_F32R else ap

    x = nc.dram_tensor("x", [BPC, H, N, N], f32, kind="ExternalInput")
    ident = nc.dram_tensor("ident", [P, P], f32, kind="ExternalInput")
    y = nc.dram_tensor("y", [BPC, N, N], bf16, kind="ExternalOutput")

    with tile.TileContext(nc) as tc:
        with (
            tc.tile_pool(name="mh", bufs=3) as mh_pool,
            tc.tile_pool(name="mr", bufs=2) as mr_pool,
            tc.tile_pool(name="agg", bufs=2) as agg_pool,
            tc.tile_pool(name="accA", bufs=4, space="PSUM") as accA_pool,
            tc.tile_pool(name="dg", bufs=4) as dg_pool,
            tc.tile_pool(name="scr", bufs=2) as scr_pool,
            tc.tile_pool(name="outp", bufs=2) as out_pool,
            tc.tile_pool(name="small", bufs=4) as small_pool,
            tc.tile_pool(name="consts", bufs=1) as const_pool,
        ):
            ident_sb = const_pool.tile([P, P], f32)
            nc.sync.dma_start(out=ident_sb, in_=ident[:, :])
            ones_sb = const_pool.tile([P, P], f32)
            nc.vector.memset(ones_sb, 1.0)

            def rowsums_act(mh, rs, m, nheads=2):
                """ACT copy+accum rowsums for heads [m, m+nheads)."""
                for mm in range(m, m + nheads):
                    scr = scr_pool.tile([P, N], f32, tag="scr")
                    for s in range(SLOTS):
                        nc.scalar.activation(
                            out=scr, in_=mh[:, mm, s, :], func=AF.Copy,
                            bias=0.0, scale=1.0,
                            accum_out=rs[:, mm, s:s + 1])

            def rowsums_dve(mh, rs, m, nheads=2):
                for mm in range(m, m + nheads):
                    for s in range(SLOTS):
                        nc.vector.tensor_reduce(
                            out=rs[:, mm, s:s + 1], in_=mh[:, mm, s, :],
                            axis=AX, op=OP.add)

            def rem_unit(h, mr_m, rsr_m, agg_rem):
                """Remainder rows for head h: ACT rowsum (copy+accum),
                then GPSIMD scalar_tensor_tensor accumulate."""
                scr = scr_pool.tile([P, N], f32, tag="scr")
                nc.scalar.activation(out=scr[0:REMP, :], in_=mr_m,
                                     func=AF.Copy, bias=0.0, scale=1.0,
                                     accum_out=rsr_m)
                if h == 0:
                    nc.vector.tensor_scalar_mul(out=agg_rem, in0=mr_m,
                                                scalar1=rsr_m)
                else:
                    sc2 = scr_pool.tile([REMP, N], f32, tag="scr2", bufs=2)
                    nc.scalar.activation(out=sc2, in_=mr_m, func=AF.Copy,
                                         bias=0.0, scale=rsr_m)
                    nc.gpsimd.tensor_tensor(out=agg_rem, in0=sc2,
                                            in1=agg_rem, op=OP.add)

            def unit(h, s, mh_s, rs_s, agg, accs):
                """Scale-accumulate head h's slot s: agg_s += rs*m."""
                if s in PE_SLOTS:
                    dg = dg_pool.tile([P, P], f32, tag="dg")
                    nc.vector.tensor_scalar_mul(out=dg, in0=ident_sb,
                                                scalar1=rs_s)
                    acc = accs[s]
                    for j0, jn in JSPLITS:
                        nc.tensor.matmul(acc[:, j0:j0 + jn],
                                         lhsT=mmview(dg),
                                         rhs=mmview(mh_s[:, j0:j0 + jn]),
                                         start=(h == 0), stop=(h == H - 1))
                    return
                a = agg[:, s, :]
                if h == 0:
                    nc.vector.tensor_scalar_mul(out=a, in0=mh_s, scalar1=rs_s)
                else:
                    nc.vector.scalar_tensor_tensor(
                        out=a, in0=mh_s, scalar=rs_s, in1=a,
                        op0=OP.mult, op1=OP.add)

            def finalize_slot(s, agg, accs, nm, sums):
                """Online softmax for one finished slot: negated max + exp."""
                src = accs[s][:, 0:N] if s in PE_SLOTS else agg[:, s, :]
                nc.vector.tensor_reduce(out=nm[:, s:s + 1], in_=src, axis=AX,
                                        op=OP.max, negate=True)
                nc.scalar.activation(out=agg[:, s, :], in_=src, func=AF.Exp,
                                     bias=nm[:, s:s + 1], scale=1.0,
                                     accum_out=sums[:, s:s + 1])

            def finalize_rem(agg_rem, nmr, sumr):
                nc.vector.tensor_reduce(out=nmr, in_=agg_rem, axis=AX,
                                        op=OP.max, negate=True)
                nc.scalar.activation(out=agg_rem, in_=agg_rem, func=AF.Exp,
                                     bias=nmr, scale=1.0, accum_out=sumr)

            for b in range(BPC):
                tailb = b == BPC - 1
                agg = agg_pool.tile([P, SLOTS, N], f32, tag="agg")
                agg_rem = small_pool.tile([REMP, N], f32, tag="aggr", bufs=2)
                nm = small_pool.tile([P, SLOTS], f32, tag="nm", bufs=2)
                nmr = small_pool.tile([REMP, 1], f32, tag="nmr", bufs=2)
                sums = small_pool.tile([P, SLOTS], f32, tag="sums", bufs=2)
                sumr = small_pool.tile([REMP, 1], f32, tag="sumr", bufs=2)
                accs = [accA_pool.tile([P, 1024], f32, tag="acc",
                                       name=f"acc_{b}_{s}")
                        for s in range(3)]

                npairs = H // 2 - 1 if tailb else H // 2
                for hp in range(npairs):
                    mh = mh_pool.tile([P, 2, SLOTS, N], f32, tag="mh")
                    src = x[b, 2 * hp:2 * hp + 2, 0:P * SLOTS, :].rearrange(
                        "h (p r) j -> p h r j", p=P)
                    nc.sync.dma_start(out=mh, in_=src)
                    mr = mr_pool.tile([REMP, 2, N], f32, tag="mr")
                    srcr = x[b, 2 * hp:2 * hp + 2, P * SLOTS:N, :].rearrange(
                        "h p j -> p h j")
                    nc.scalar.dma_start(out=mr, in_=srcr)

                    rs = small_pool.tile([P, 2, SLOTS], f32, tag="rs", bufs=4)
                    rsr = small_pool.tile([REMP, 2], f32, tag="rsrw", bufs=4)
                    if hp in RS_ACT_PAIRS:
                        rowsums_act(mh, rs, 0)
                    else:
                        rowsums_dve(mh, rs, 0)
                    for m in range(2):
                        h = 2 * hp + m
                        rem_unit(h, mr[:, m, :], rsr[:, m:m + 1], agg_rem)
                        for s in range(SLOTS):
                            unit(h, s, mh[:, m, s, :], rs[:, m, s:s + 1],
                                 agg, accs)
                        if h == H - 1:
                            finalize_rem(agg_rem, nmr, sumr)
                            for s in range(SLOTS):
                                finalize_slot(s, agg, accs, nm, sums)

                m1p = small_pool.tile([P, 1], f32, tag="m1p", bufs=2)
                if tailb:
                    # head H-2: single full load
                    mh = mh_pool.tile([P, 1, SLOTS, N], f32, tag="mh")
                    src = x[b, H - 2, 0:P * SLOTS, :].rearrange(
                        "(p r) j -> p r j", p=P)
                    nc.sync.dma_start(out=mh[:, 0, :, :], in_=src)
                    mr = mr_pool.tile([REMP, 2, N], f32, tag="mr")
                    nc.scalar.dma_start(out=mr[:, 0, :],
                                        in_=x[b, H - 2, P * SLOTS:N, :])
                    rs = small_pool.tile([P, 2, SLOTS], f32, tag="rs", bufs=4)
                    rsr = small_pool.tile([REMP, 2], f32, tag="rsrw", bufs=4)
                    rowsums_act(mh, rs, 0, nheads=1)
                    rem_unit(H - 2, mr[:, 0, :], rsr[:, 0:1], agg_rem)
                    for s in range(SLOTS):
                        unit(H - 2, s, mh[:, 0, s, :], rs[:, 0, s:s + 1],
                             agg, accs)

                    # head H-1: remainder first, then [2,2,1,1] slot pieces
                    nc.scalar.dma_start(out=mr[:, 1, :],
                                        in_=x[b, H - 1, P * SLOTS:N, :])
                    rem_unit(H - 1, mr[:, 1, :], rsr[:, 1:2], agg_rem)
                    finalize_rem(agg_rem, nmr, sumr)

                    mh = mh_pool.tile([P, 1, SLOTS, N], f32, tag="mh")
                    rs2 = small_pool.tile([P, 2, SLOTS], f32, tag="rs", bufs=4)
                    src = x[b, H - 1, 0:P * SLOTS, :].rearrange(
                        "(p r) j -> p r j", p=P)
                    for piece in ((0, 1), (2, 3), (4,), (5,)):
                        sl = slice(piece[0], piece[-1] + 1)
                        nc.sync.dma_start(out=mh[:, 0, sl, :],
                                          in_=src[:, sl, :])
                        for s in piece:
                            nc.vector.tensor_reduce(out=rs2[:, 0, s:s + 1],
                                                    in_=mh[:, 0, s, :],
                                                    axis=AX, op=OP.add)
                        for s in piece:
                            unit(H - 1, s, mh[:, 0, s, :], rs2[:, 0, s:s + 1],
                                 agg, accs)
                            finalize_slot(s, agg, accs, nm, sums)
                        if piece == (4,):
                            # partial global-min over slots 0-4 + remainder
                            nc.vector.tensor_reduce(out=m1p, in_=nm[:, 0:5],
                                                    axis=AX, op=OP.min)
                            nc.vector.tensor_tensor(out=m1p[0:REMP, :],
                                                    in0=m1p[0:REMP, :],
                                                    in1=nmr, op=OP.min)

                # ---- global softmax correction chain for this batch ----
                m1 = small_pool.tile([P, 1], f32, tag="m1", bufs=2)
                if tailb:
                    nc.vector.tensor_tensor(out=m1, in0=m1p, in1=nm[:, 5:6],
                                            op=OP.min)
                else:
                    nc.vector.tensor_reduce(out=m1, in_=nm, axis=AX, op=OP.min)
                    nc.vector.tensor_tensor(out=m1[0:REMP, :],
                                            in0=m1[0:REMP, :],
                                            in1=nmr, op=OP.min)
                tps = accA_pool.tile([1, P], f32, tag="acc", name=f"tps_{b}")
                nc.tensor.transpose(tps, m1, ident_sb)
                gmn = small_pool.tile([1, 1], f32, tag="gmn", bufs=2)
                nc.vector.tensor_reduce(out=gmn, in_=tps, axis=AX, op=OP.min)
                bps = accA_pool.tile([P, 1], f32, tag="acc", name=f"bps_{b}")
                nc.tensor.matmul(bps, lhsT=ones_sb[0:1, :], rhs=gmn,
                                 start=True, stop=True)
                negM = small_pool.tile([P, 1], f32, tag="negM", bufs=2)
                nc.vector.tensor_copy(out=negM, in_=bps)
                cfac = small_pool.tile([P, SLOTS], f32, tag="cfac", bufs=2)
                nc.scalar.activation(out=cfac, in_=nm, func=AF.Exp,
                                     bias=negM, scale=-1.0)
                cfr = small_pool.tile([REMP, 1], f32, tag="cfr", bufs=2)
                nc.scalar.activation(out=cfr, in_=nmr, func=AF.Exp,
                                     bias=negM[0:REMP, :], scale=-1.0)
                zc = small_pool.tile([P, SLOTS], f32, tag="zc", bufs=2)
                nc.vector.tensor_tensor(out=zc, in0=sums, in1=cfac, op=OP.mult)
                z1 = small_pool.tile([P, 1], f32, tag="z1", bufs=2)
                nc.vector.tensor_reduce(out=z1, in_=zc, axis=AX, op=OP.add)
                zr = small_pool.tile([REMP, 1], f32, tag="zr", bufs=2)
                nc.vector.tensor_tensor(out=zr, in0=sumr, in1=cfr, op=OP.mult)
                nc.vector.tensor_tensor(out=z1[0:REMP, :], in0=z1[0:REMP, :],
                                        in1=zr, op=OP.add)
                sps = accA_pool.tile([P, 1], f32, tag="acc", name=f"sps_{b}")
                nc.tensor.matmul(sps, lhsT=ones_sb, rhs=z1, start=True,
                                 stop=True)
                rinv = small_pool.tile([P, 1], f32, tag="rinv", bufs=2)
                nc.vector.reciprocal(out=rinv, in_=sps)
                f = small_pool.tile([P, SLOTS], f32, tag="f", bufs=2)
                nc.vector.tensor_scalar_mul(out=f, in0=cfac, scalar1=rinv)
                fr = small_pool.tile([REMP, 1], f32, tag="fr", bufs=2)
                nc.vector.tensor_scalar_mul(out=fr, in0=cfr,
                                            scalar1=rinv[0:REMP, :])

                # ---- final scale (+bf16 cast) and store ----
                dst = y[b, 0:P * SLOTS, :].rearrange("(p r) j -> p r j", p=P)
                ot = out_pool.tile([P, SLOTS, N], bf16, tag="out", bufs=1)
                for g0, q in ((0, nc.sync), (2, nc.scalar), (4, nc.sync)):
                    for k in range(2):
                        s = g0 + k
                        if k == 0:
                            nc.scalar.activation(out=ot[:, s, :],
                                                 in_=agg[:, s, :],
                                                 func=AF.Copy, bias=0.0,
                                                 scale=f[:, s:s + 1])
                        else:
                            nc.vector.tensor_scalar_mul(out=ot[:, s, :],
                                                        in0=agg[:, s, :],
                                                        scalar1=f[:, s:s + 1])
                    q.dma_start(out=dst[:, g0:g0 + 2, :],
                                in_=ot[:, g0:g0 + 2, :])
                otr = out_pool.tile([REMP, N], bf16, tag="outr", bufs=2)
                nc.vector.tensor_scalar_mul(out=otr, in0=agg_rem, scalar1=fr)
                nc.scalar.dma_start(out=y[b, P * SLOTS:N, :], in_=otr)

    nc.finalize()
    return nc


def kernel(mha_masks) -> np.ndarray:
    global LAST_RESULT
    from concourse.bass_utils import run_bass_kernel_spmd

    xfull = np.ascontiguousarray(np.asarray(mha_masks, dtype=np.float32))
    assert xfull.shape == (B, H, N, N), xfull.shape

    nc = build_program()
    ident = np.eye(P, dtype=np.float32)
    in_maps = [
        {"x": xfull[i * BPC: (i + 1) * BPC], "ident": ident}
        for i in range(NCORES)
    ]
    import os

    kw = {}
    if os.environ.get("KERNEL_TRACE_DIR"):
        kw = dict(trace=True, tmpdir=os.environ["KERNEL_TRACE_DIR"])
    res = run_bass_kernel_spmd(nc, in_maps, core_ids=list(range(NCORES)), **kw)
    LAST_RESULT = res
    out = np.concatenate(
        [np.asarray(r["y"]).astype(np.float32) for r in res.results], axis=0
    )
    return out
